# revision 20
# baseline (speedup 1.0000x reference)
"""Trainium2 Bass kernel for nn_ContrastiveModel (ColBERT-style MaxSim scoring).

score[b] = (sum_i max_j cos(a1[b,i], a2[b,j]) + sum_j max_i cos(...)) / (n1+n2)
with prefix validity masks (pos < sum(att_mask)).

v5 strategy (pure data parallel, 8 batches per core):
  - gpsimd casting DMA: HBM f32 -> SBUF bf16 in flight (frees ACT/DVE from
    the f32 cast pass entirely; conversion is free in the SDMA datapath)
  - flat contiguous layout: SBUF [128, 4, 768], token(p,t) = 4p+t
  - norms from the bf16 data: ACT Square+accum per t-chunk; w = valid/norm
    via sqrt (ACT) + reciprocal (DVE) + validity TT
  - scale pass in bf16 (DVE tensor_scalar, 4x_2p mode: 4 elem/cycle)
  - transpose to [d, token] via PE transpose-mode, bf16 PSUM, merged
    2x-mode copies (4 DVE / 2 ACT per batch)
  - main matmul bf16 into paired PSUM banks [128,2,512]; two-at-once
    row-max reduces; fold tree (TT of PSUM pairs, then bf16 2x TT);
    col max via PE matmul vs identity + one reduce
  - software pipelined: prep(b+1) before compute(b); marshal groups of b+1
    zippered between main-matmul groups of b (keeps PE warm for HAM)
  - epilogue: ones-matmul partition sums, scores = total * 1/(n1+n2)
"""

import os
import sys

sys.path.insert(0, "/opt/trn_rl_repo")

import numpy as np
import ml_dtypes
from contextlib import ExitStack

import concourse.bacc as bacc
import concourse.bass as bass
import concourse.tile as tile
from concourse import mybir
from concourse import bass_utils
from concourse._compat import with_exitstack


def _axon_device_reset():
    """The terminal-side accelerator is often left unrecoverable by a previous
    client process; axon_reset clears that state. Call before first use."""
    import ctypes
    try:
        lib = ctypes.CDLL("/opt/axon/libaxon_pjrt.so")
        lib.axon_reset.restype = ctypes.c_int64
        rc = lib.axon_reset()
        if rc != 0:
            print("axon_reset rc:", rc)
    except Exception as e:
        print("axon_reset failed:", e)


_axon_device_reset()

N_CORES = 8
B_FULL, S, D = 64, 512, 768
BPC = B_FULL // N_CORES  # batches per core
NT = S // 128  # token blocks (4)
NK = D // 128  # d blocks (6)

F32 = mybir.dt.float32
BF16 = mybir.dt.bfloat16
I32 = mybir.dt.int32
AX = mybir.AxisListType
ALU = mybir.AluOpType
ACTF = mybir.ActivationFunctionType


@with_exitstack
def _emit(ctx: ExitStack, tc: tile.TileContext, aps: dict):
    nc = tc.nc

    # flat contiguous load: partition p holds tokens 4p..4p+3
    a1r = aps["a1"].rearrange("b (p x) d -> b p (x d)", p=128)
    a2r = aps["a2"].rearrange("b (p x) d -> b p (x d)", p=128)

    consts = ctx.enter_context(tc.tile_pool(name="consts", bufs=1))
    craw = ctx.enter_context(tc.tile_pool(name="craw", bufs=4))
    csc = ctx.enter_context(tc.tile_pool(name="csc", bufs=3))
    sq = ctx.enter_context(tc.tile_pool(name="sq", bufs=2))
    small = ctx.enter_context(tc.tile_pool(name="small", bufs=2))
    tT = ctx.enter_context(tc.tile_pool(name="tT", bufs=2))
    fold = ctx.enter_context(tc.tile_pool(name="fold", bufs=3))
    psS = ctx.enter_context(tc.tile_pool(name="psS", bufs=4, space="PSUM"))
    psT = ctx.enter_context(tc.tile_pool(name="psT", bufs=3, space="PSUM"))
    psE = ctx.enter_context(tc.tile_pool(name="psE", bufs=1, space="PSUM"))

    # ---- constants ----
    IDB = consts.tile([128, 128], BF16, tag="idb")
    nc.sync.dma_start(out=IDB[:], in_=aps["idb"][:])
    IDF = consts.tile([128, 128], F32, tag="idf")
    nc.sync.dma_start(out=IDF[:], in_=aps["idf"][:])
    IOTA = consts.tile([128, NT], F32, tag="iota")
    nc.sync.dma_start(out=IOTA[:], in_=aps["iota"][:])
    ONES = consts.tile([128, 1], F32, tag="ones")
    nc.vector.memset(ONES[:], 1.0)

    # ---- masks -> n1, n2, 1/(n1+n2), broadcast n to all partitions ----
    M1i = consts.tile([BPC, S], I32, tag="m1i")
    nc.sync.dma_start(out=M1i[:], in_=aps["m1"][:])
    M2i = consts.tile([BPC, S], I32, tag="m2i")
    nc.sync.dma_start(out=M2i[:], in_=aps["m2"][:])
    M1f = consts.tile([BPC, S], F32, tag="m1f")
    nc.vector.tensor_copy(M1f[:], M1i[:])
    M2f = consts.tile([BPC, S], F32, tag="m2f")
    nc.vector.tensor_copy(M2f[:], M2i[:])
    n1 = consts.tile([BPC, 1], F32, tag="n1")
    nc.vector.tensor_reduce(out=n1[:], in_=M1f[:], axis=AX.X, op=ALU.add)
    n2 = consts.tile([BPC, 1], F32, tag="n2")
    nc.vector.tensor_reduce(out=n2[:], in_=M2f[:], axis=AX.X, op=ALU.add)
    ns = consts.tile([BPC, 1], F32, tag="ns")
    nc.vector.tensor_add(ns[:], n1[:], n2[:])
    rns = consts.tile([BPC, 1], F32, tag="rns")
    nc.vector.reciprocal(rns[:], ns[:])

    # spread n1/n2 to the free dim: [BPC,1] -> [1,BPC] via PE, then broadcast
    pe_n1 = psE.tile([128, NT, 128], F32, tag="eps")
    nc.tensor.matmul(out=pe_n1[:1, 0, :BPC], lhsT=n1[:], rhs=IDF[:BPC, :BPC],
                     start=True, stop=True)
    N1r = consts.tile([1, BPC], F32, tag="n1r")
    nc.vector.tensor_copy(N1r[:], pe_n1[:1, 0, :BPC])
    pe_n2 = psE.tile([128, NT, 128], F32, tag="eps")
    nc.tensor.matmul(out=pe_n2[:1, 0, :BPC], lhsT=n2[:], rhs=IDF[:BPC, :BPC],
                     start=True, stop=True)
    N2r = consts.tile([1, BPC], F32, tag="n2r")
    nc.vector.tensor_copy(N2r[:], pe_n2[:1, 0, :BPC])
    # broadcast to all partitions via K=1 ones-matmul
    ONESR = consts.tile([1, 128], F32, tag="onesr")
    nc.vector.memset(ONESR[:], 1.0)
    pe_b1 = psE.tile([128, NT, 128], F32, tag="eps")
    nc.tensor.matmul(out=pe_b1[:, 0, :BPC], lhsT=ONESR[:], rhs=N1r[:],
                     start=True, stop=True)
    N1B = consts.tile([128, BPC], F32, tag="n1b")
    nc.vector.tensor_copy(N1B[:], pe_b1[:, 0, :BPC])
    pe_b2 = psE.tile([128, NT, 128], F32, tag="eps")
    nc.tensor.matmul(out=pe_b2[:, 0, :BPC], lhsT=ONESR[:], rhs=N2r[:],
                     start=True, stop=True)
    N2B = consts.tile([128, BPC], F32, tag="n2b")
    nc.vector.tensor_copy(N2B[:], pe_b2[:, 0, :BPC])

    # result collector: 8 columns per batch (4 row-max blocks + 4 col-max blocks)
    RC = consts.tile([128, BPC * 8], F32, tag="rc")

    # ---- HAM warmup: keep PE busy until the first marshal matmuls arrive
    # (~15us in) so the clock is at 8/8 and no MID-window idle re-throttles ----
    WUP = psS.tile([128, S], F32, tag="sim")
    for i in range(40):
        nc.tensor.matmul(out=WUP[:, :128], lhsT=IDB[:], rhs=IDB[:],
                         start=True, stop=True)

    def emit_dma(b, split=False):
        """casting DMA: HBM f32 -> SBUF bf16 (SWDGE converts in flight)"""
        out = []
        for r, tag in ((a1r, "c1"), (a2r, "c2")):
            C = craw.tile([128, NT, D], BF16, tag=tag)
            f = C.rearrange("p t d -> p (t d)")
            if split:
                h = (NT // 2) * D
                nc.gpsimd.dma_start(out=f[:, :h], in_=r[b][:, :h])
                nc.gpsimd.dma_start(out=f[:, h:], in_=r[b][:, h:])
            else:
                nc.gpsimd.dma_start(out=f[:], in_=r[b])
            out.append(C)
        return tuple(out)

    def emit_prep(b, C1, C2):
        """norms from bf16 -> w = valid/norm -> bf16 4x scale pass.
        For b=0 the chain runs per tensor so T1's marshal unblocks before C2
        lands; later batches use one merged chain (fewer small instructions)."""
        NSQ = small.tile([128, 2 * NT], F32, tag="nsq")
        WN = small.tile([128, 2 * NT], F32, tag="wn")
        S1 = csc.tile([128, NT, D], BF16, tag="s1")
        S2 = csc.tile([128, NT, D], BF16, tag="s2")

        def norm_chain(parts):
            for x, C, Sc, NB in parts:
                for t in range(NT):
                    s = sq.tile([128, D], BF16, tag="sq")
                    nc.scalar.activation(out=s[:], in_=C[:, t, :],
                                         func=ACTF.Square,
                                         accum_out=NSQ[:, x * NT + t: x * NT + t + 1])
            c0 = parts[0][0] * NT
            cols = slice(c0, c0 + len(parts) * NT)
            SQN = small.tile([128, len(parts) * NT], F32, tag=f"sqn{c0}")
            nc.scalar.activation(out=SQN[:], in_=NSQ[:, cols], func=ACTF.Sqrt)
            RN = small.tile([128, len(parts) * NT], F32, tag=f"rn{c0}")
            nc.vector.reciprocal(RN[:], SQN[:])
            VV = small.tile([128, len(parts) * NT], F32, tag=f"vv{c0}")
            for j, (x, C, Sc, NB) in enumerate(parts):
                nc.vector.tensor_scalar(out=VV[:, j * NT:(j + 1) * NT],
                                        in0=IOTA[:], scalar1=NB[:, b: b + 1],
                                        scalar2=None, op0=ALU.is_lt)
            nc.vector.tensor_tensor(out=WN[:, cols], in0=RN[:], in1=VV[:],
                                    op=ALU.mult)
            for x, C, Sc, NB in parts:
                for t in range(NT):
                    nc.vector.tensor_scalar(out=Sc[:, t, :], in0=C[:, t, :],
                                            scalar1=WN[:, x * NT + t: x * NT + t + 1],
                                            scalar2=None, op0=ALU.mult)

        parts = [(0, C1, S1, N1B), (1, C2, S2, N2B)]
        if b == 0:
            norm_chain(parts[:1])
            norm_chain(parts[1:])
        else:
            norm_chain(parts)
        T1 = tT.tile([128, NK, S], BF16, tag="t1")
        T2 = tT.tile([128, NK, S], BF16, tag="t2")
        return S1, S2, T1, T2

    def marshal_group(g, S1, S2, T1, T2):
        """group g in 0..5: two d-blocks of one tensor -> one PSUM bank ->
        one merged bf16 copy (2x mode)"""
        x, kp = divmod(g, NK // 2)
        C, T = (S1, T1) if x == 0 else (S2, T2)
        PT = psT.tile([128, 2, S], BF16, tag="pt")
        for h in range(2):
            k = 2 * kp + h
            for t in range(NT):
                nc.tensor.transpose(out=PT[:, h, 128 * t: 128 * (t + 1)],
                                    in_=C[:, t, 128 * k: 128 * (k + 1)],
                                    identity=IDB[:])
        dst = T[:, 2 * kp: 2 * kp + 2, :].rearrange("p a b -> p (a b)")
        src = PT.rearrange("p a b -> p (a b)")
        if g == 0:
            nc.scalar.copy(dst, src)
        else:
            nc.vector.tensor_copy(dst, src)

    def emit_compute(b, T1, T2, nxt):
        """main matmuls + reductions for batch b, with marshal groups of the
        next batch (nxt = (S1, S2, T1n, T2n) or None) zippered between"""
        groups = list(range(NK)) if nxt is not None else []
        halves = []
        for tp in range(2):
            FS = fold.tile([128, S], BF16, tag=f"fs{tp}")
            banks = []
            for tt in range(2):
                t = 2 * tp + tt
                SIMt = psS.tile([128, S], F32, tag="sim")
                banks.append(SIMt)
                # zipper: marshal groups of the next batch before each main
                for _ in range((3, 2, 1, 0)[t]):
                    if groups:
                        marshal_group(groups.pop(0), *nxt)
                for k in range(NK):
                    nc.tensor.matmul(out=SIMt[:],
                                     lhsT=T1[:, k, 128 * t: 128 * (t + 1)],
                                     rhs=T2[:, k, :], start=(k == 0),
                                     stop=(k == NK - 1))
                if tt == 0:
                    # fold seed (ACT) overlaps the second bank's matmuls
                    nc.scalar.copy(FS[:], SIMt[:])
            # single-bank row maxes; bank 0's reduce frees it for the next
            # batch without waiting on the fold TT
            nc.vector.tensor_reduce(out=RC[:, 8 * b + 2 * tp: 8 * b + 2 * tp + 1],
                                    in_=banks[0][:], axis=AX.X, op=ALU.max)
            FH = fold.tile([128, S], BF16, tag=f"fh{tp}")
            nc.vector.tensor_tensor(out=FH[:], in0=banks[1][:],
                                    in1=FS[:], op=ALU.max)
            nc.vector.tensor_reduce(out=RC[:, 8 * b + 2 * tp + 1: 8 * b + 2 * tp + 2],
                                    in_=banks[1][:], axis=AX.X, op=ALU.max)
            halves.append(FH)
        for g in groups:
            marshal_group(g, *nxt)
        F = fold.tile([128, S], BF16, tag="f")
        nc.vector.tensor_tensor(out=F[:], in0=halves[0][:], in1=halves[1][:],
                                op=ALU.max)
        # col max: PE matmul of the fold vs identity, then free-dim reduce
        PF = psE.tile([128, NT, 128], F32, tag="eps")
        for t in range(NT):
            nc.tensor.matmul(out=PF[:, t, :],
                             lhsT=F[:, 128 * t: 128 * (t + 1)],
                             rhs=IDB[:], start=True, stop=True)
        nc.vector.tensor_reduce(out=RC[:, 8 * b + NT: 8 * b + 8], in_=PF[:],
                                axis=AX.X, op=ALU.max)

    # ---- software-pipelined batch loop ----
    A = {}
    A[0] = emit_dma(0, split=True)
    A[1] = emit_dma(1)
    prep = {0: emit_prep(0, *A[0])}
    for g in range(NK):
        marshal_group(g, *prep[0])
    for b in range(BPC):
        if b + 2 < BPC:
            A[b + 2] = emit_dma(b + 2)
        if b + 1 < BPC:
            prep[b + 1] = emit_prep(b + 1, *A[b + 1])
        S1, S2, T1, T2 = prep[b]
        emit_compute(b, T1, T2, prep.get(b + 1))
        del prep[b], A[b]

    # ---- epilogue: scores = (sum over RC columns, grouped by batch) / (n1+n2) ----
    pe_sum = psE.tile([128, NT, 128], F32, tag="eps")
    nc.tensor.matmul(out=pe_sum[:1, 0, : BPC * 8], lhsT=ONES[:], rhs=RC[:],
                     start=True, stop=True)
    SC = consts.tile([1, BPC * 8], F32, tag="sc")
    nc.vector.tensor_copy(SC[:], pe_sum[:1, 0, : BPC * 8])
    TOT = consts.tile([1, BPC], F32, tag="tot")
    nc.vector.tensor_reduce(out=TOT[:], in_=SC.rearrange("p (g x) -> p g x", x=8),
                            axis=AX.X, op=ALU.add)
    pe_t = psE.tile([128, NT, 128], F32, tag="eps")
    nc.tensor.matmul(out=pe_t[:BPC, 0, :1], lhsT=TOT[:], rhs=ONES[:1, :1],
                     start=True, stop=True)
    SCT = consts.tile([BPC, 1], F32, tag="sct")
    nc.vector.tensor_copy(SCT[:], pe_t[:BPC, 0, :1])
    OUT = consts.tile([BPC, 1], F32, tag="out")
    nc.vector.tensor_mul(OUT[:], SCT[:], rns[:])
    nc.sync.dma_start(out=aps["out"][:], in_=OUT[:])


_CACHE = {}


def _patch_ldw_opt():
    """concourse compiles with --enable-ldw-opt=false; marshal transposes are
    LDWEIGHTS-bound (107ns/block) without it. Rewrite the walrus argv."""
    if getattr(bass_utils, "_ldw_patched", False):
        return
    orig = bass_utils.run_command

    def patched(argv, **kw):
        argv = [a.replace("--enable-ldw-opt=false", "--enable-ldw-opt=true")
                if isinstance(a, str) else a for a in argv]
        return orig(argv, **kw)

    bass_utils.run_command = patched
    bass_utils._ldw_patched = True


def _build():
    if "nc" in _CACHE:
        return _CACHE["nc"]
    if "--ldw-opt" in sys.argv or os.environ.get("LDW_OPT"):
        _patch_ldw_opt()
    nc = bacc.Bacc("TRN2", target_bir_lowering=False, debug=False,
                   num_devices=N_CORES)
    aps = {
        "a1": nc.dram_tensor("a1", [BPC, S, D], F32, kind="ExternalInput").ap(),
        "a2": nc.dram_tensor("a2", [BPC, S, D], F32, kind="ExternalInput").ap(),
        "m1": nc.dram_tensor("m1", [BPC, S], I32, kind="ExternalInput").ap(),
        "m2": nc.dram_tensor("m2", [BPC, S], I32, kind="ExternalInput").ap(),
        "idb": nc.dram_tensor("idb", [128, 128], BF16, kind="ExternalInput").ap(),
        "idf": nc.dram_tensor("idf", [128, 128], F32, kind="ExternalInput").ap(),
        "iota": nc.dram_tensor("iota", [128, NT], F32, kind="ExternalInput").ap(),
        "out": nc.dram_tensor("out", [BPC, 1], F32, kind="ExternalOutput").ap(),
    }
    with tile.TileContext(nc) as tc:
        _emit(tc, aps)
    nc.compile()
    _CACHE["nc"] = nc
    return nc


def _consts():
    return {
        "idb": np.eye(128, dtype=ml_dtypes.bfloat16),
        "idf": np.eye(128, dtype=np.float32),
        # token(p, t) = 4p + t  (flat contiguous DMA mapping)
        "iota": (4.0 * np.arange(128, dtype=np.float32)[:, None]
                 + np.arange(NT, dtype=np.float32)[None, :]),
    }


def make_in_maps(article_1_emb, article_2_emb, article_1_att_mask,
                 article_2_att_mask):
    a1 = np.ascontiguousarray(np.asarray(article_1_emb, dtype=np.float32))
    a2 = np.ascontiguousarray(np.asarray(article_2_emb, dtype=np.float32))
    m1 = np.ascontiguousarray(np.asarray(article_1_att_mask, dtype=np.int32))
    m2 = np.ascontiguousarray(np.asarray(article_2_att_mask, dtype=np.int32))
    cst = _consts()
    in_maps = []
    for c in range(N_CORES):
        sl = slice(c * BPC, (c + 1) * BPC)
        in_maps.append({"a1": a1[sl], "a2": a2[sl], "m1": m1[sl], "m2": m2[sl],
                        **cst})
    return in_maps


def _ensure_profile_hook():
    """bass_utils' axon trace path imports antenv.axon_hooks, which this
    image lacks. Inject it and register the ctypes NTFF hook."""
    import types

    if "antenv.axon_hooks" in sys.modules:
        return
    mod = types.ModuleType("antenv.axon_hooks")
    mod._hook = None
    mod.set_axon_ntff_profile_hook = lambda h: setattr(mod, "_hook", h)
    mod.get_axon_ntff_profile_hook = lambda: mod._hook
    sys.modules["antenv.axon_hooks"] = mod
    try:
        from trn_agent_boot.trn_boot import _ntff_profile_via_ctypes
        mod._hook = _ntff_profile_via_ctypes("/opt/axon/libaxon_pjrt.so")
    except Exception as e:
        print("ntff hook setup failed:", e)


def kernel(article_1_emb, article_2_emb, article_1_att_mask,
           article_2_att_mask, _trace=False, _trace_kwargs=None):
    if _trace:
        _ensure_profile_hook()
    nc = _build()
    in_maps = make_in_maps(article_1_emb, article_2_emb, article_1_att_mask,
                           article_2_att_mask)
    res = bass_utils.run_bass_kernel_spmd(
        nc, in_maps, core_ids=list(range(N_CORES)), trace=_trace,
        **(_trace_kwargs or {}))
    out = np.concatenate([np.asarray(res.results[c]["out"]).reshape(BPC)
                          for c in range(N_CORES)])
    if _trace:
        return out.astype(np.float32), res
    return out.astype(np.float32)


if __name__ == "__main__":
    # quick CoreSim check against numpy for core 0's slice
    rng = np.random.default_rng(0)
    a1 = rng.standard_normal((BPC, S, D), dtype=np.float32)
    a2 = rng.standard_normal((BPC, S, D), dtype=np.float32)
    m1 = rng.integers(0, 2, size=(BPC, S)).astype(np.int32)
    m2 = rng.integers(0, 2, size=(BPC, S)).astype(np.int32)

    nc = _build()
    print("compiled ok", flush=True)

    from concourse.bass_interp import CoreSim
    sim = CoreSim(nc)
    cst = _consts()
    for k, v in (("a1", a1), ("a2", a2), ("m1", m1), ("m2", m2), *cst.items()):
        sim.tensor(k)[:] = v
    sim.simulate()
    got = np.asarray(sim.tensor("out")).reshape(BPC)

    n1 = m1.sum(-1); n2 = m2.sum(-1)
    pos = np.arange(S)
    w1 = (pos[None, :] < n1[:, None]) / np.linalg.norm(a1, axis=-1)
    w2 = (pos[None, :] < n2[:, None]) / np.linalg.norm(a2, axis=-1)
    M = np.einsum("bid,bjd->bij", a1 * w1[..., None], a2 * w2[..., None])
    want = (M.max(2).sum(-1) + M.max(1).sum(-1)) / (n1 + n2)
    print("sim:", got)
    print("ref:", want)
    print("rel err:", np.abs(got - want).max() / np.abs(want).max())
